# revision 42
# baseline (speedup 1.0000x reference)
"""Trainium2 Bass kernel for nn_Attention_13348758356565.

Dense attention block (B=16, N=1024, DIM=1024, 16 heads x 64) with axial
rotary embeddings, data-parallel over batch across 8 NeuronCores (2 batches
per core). ~543us HW vs ~1288us f32r baseline (2.4x), rel_absmax ~8.5e-3.

Design notes (hardware-measured, CoreSim cost model diverges):
- All matmul operands bf16: HW streams fp32/f32r moving operands at ~2
  cyc/col (4B bandwidth limit) but bf16 at 1 cyc/col (~216ns per 512-col
  MM warm) => 2x PE throughput vs the f32r baseline.
- QK^T packed as 2-head row-group pairs (stationary at base partitions
  0/64, K=64): the two MMs run concurrently on the PE array (4ns apart).
- One exp ACTIVATE per [128,1024] score pair spanning 2 PSUM banks
  (matmul dsts stay per-bank; ACT reads may cross banks) -> ~1025ns vs
  2x824ns split.
- Softmax denominators ride a 65th ones-column in the PV stationary (any
  separate denominator pass costs exactly the stream time it would save).
- PV psum evacuation on the scalar engine (ACT Copy); reciprocal of the
  denominator row must first be copied to partition 0 (HW's
  reciprocal_approx_fast ignores a nonzero base partition - sim doesn't).
- Emission-interleaved "filler": QKV/rotary/transpose/out-proj work of the
  other batch is pumped between attention iterations so the PE never idles
  while scalar runs exp (keeps HAM at K=8/8). Trailing per-group transposes
  are carried into the next chunk's matmul stream because the TileScheduler
  hands out PSUM slots in priority order (a transpose waiting on the rotary
  DVE chain would otherwise block the next psum chain).
- Rotary on natural-layout bf16 tiles with an [evens|odds|pass] head-dim
  permutation folded into w_qkv rows host-side (packed last-dim slices ->
  DVE 2x/4x modes), emitted in two halves so its latency hides under the
  psum-evacuation CASTs; Q/K transposed to [dim, token] via PE transposes.
"""

import os
import sys

sys.path.insert(0, "/opt/trn_rl_repo")

import dataclasses
from collections import deque

import numpy as np

import concourse.bacc as bacc
import concourse.mybir as mybir
import concourse.tile as tile
from concourse import bass_utils

F32 = mybir.dt.float32
BF16 = mybir.dt.bfloat16
EXP = mybir.ActivationFunctionType.Exp
CPY = mybir.ActivationFunctionType.Copy

B, HF, WF = 16, 32, 32
DIM, NH, HD = 1024, 16, 64
N = HF * WF          # 1024 tokens
NCORES = 8
BPC = B // NCORES    # 2 batches per core
ROT = HD // 2        # 32 rotary dims per head
NT = N // 128        # 8 token tiles
ND = DIM // 128      # 8 contraction tiles
NHP = NH // 2        # 8 head pairs
SCALE = 1.0 / np.sqrt(HD)

last_exec_time_ns = None


def _freq_tables():
    d = HD // 4
    base = (np.linspace(1.0, (HF * WF) / 2.0, d // 2, dtype=np.float64) * np.pi)
    posH = np.linspace(-1.0, 1.0, HF)
    posW = np.linspace(-1.0, 1.0, WF)
    fH = np.repeat(posH[:, None] * base[None, :], 2, axis=-1)   # [H, 16]
    fW = np.repeat(posW[:, None] * base[None, :], 2, axis=-1)   # [W, 16]
    fH = np.broadcast_to(fH[:, None, :], (HF, WF, d))
    fW = np.broadcast_to(fW[None, :, :], (HF, WF, d))
    freqs = np.concatenate([fH, fW], axis=-1).reshape(N, ROT)
    # freqs[:, 2i] == freqs[:, 2i+1]; keep one per pair -> [N, 16]
    half = freqs[:, 0::2]
    return np.cos(half), np.sin(half)


def _qk_perm():
    """Per-head dim order [e0..e15, o0..o15, pass0..pass31]."""
    per_head = np.concatenate([
        np.arange(0, ROT, 2), np.arange(1, ROT, 2), np.arange(ROT, HD)])
    return (np.arange(NH)[:, None] * HD + per_head[None, :]).reshape(-1)


def _ins_bcast(ap, count, pos):
    """Insert a step-0 dim of size `count` at free-dim position `pos`."""
    dims = list(ap.ap)
    dims.insert(pos + 1, [0, count])
    return dataclasses.replace(ap, ap=dims)


def _build():
    nc = bacc.Bacc("TRN2", target_bir_lowering=False, debug=False)

    xT_d = nc.dram_tensor("xT", [BPC, DIM, N], BF16, kind="ExternalInput")
    wqkvT_d = nc.dram_tensor("wqkvT", [DIM, 3 * DIM], BF16, kind="ExternalInput")
    wprojT_d = nc.dram_tensor("wprojT", [DIM, DIM], BF16, kind="ExternalInput")
    bproj_d = nc.dram_tensor("bproj", [1, DIM], BF16, kind="ExternalInput")
    cosh_d = nc.dram_tensor("cosh", [128, NT * 16], BF16,
                            kind="ExternalInput")
    sinh_d = nc.dram_tensor("sinh", [128, NT * 16], BF16,
                            kind="ExternalInput")
    ident_d = nc.dram_tensor("ident", [128, 128], BF16, kind="ExternalInput")
    y_d = nc.dram_tensor("y", [BPC, N, DIM], F32, kind="ExternalOutput")

    mul = mybir.AluOpType.mult
    sub = mybir.AluOpType.subtract
    add = mybir.AluOpType.add

    with tile.TileContext(nc) as tc:
        with (
            tc.tile_pool(name="sb", bufs=1) as sb,
            tc.tile_pool(name="ps", bufs=1, space="PSUM") as ps,
        ):
            # ---- constants (emitted after the first input DMAs) ----
            cosh = sinh = ident = bias_bc = None

            def emit_consts():
                nonlocal cosh, sinh, ident, bias_bc
                ident = sb.tile([128, 128], BF16, name="ident")
                nc.scalar.dma_start(ident[:], ident_d.ap())
                cosh = sb.tile([128, NT * 16], BF16, name="cosh")
                sinh = sb.tile([128, NT * 16], BF16, name="sinh")
                nc.scalar.dma_start(cosh[:], cosh_d.ap())
                nc.scalar.dma_start(sinh[:], sinh_d.ap())
                bias_row = sb.tile([1, DIM], BF16, name="bias_row")
                nc.scalar.dma_start(bias_row[:], bproj_d.ap())
                bias_bc = sb.tile([128, DIM], BF16, name="bias_bc")
                nc.gpsimd.partition_broadcast(bias_bc[:], bias_row[:])

            # live tile registries (filled by closures at emission time)
            xTt, kT, qT, vsL, vsR, outT = {}, {}, {}, {}, {}, {}
            done = set()   # emission-complete markers for chunks

            filler = deque()
            pending = []   # tail closures to splice into the next chunk

            def pump(budget):
                while budget > 0 and filler:
                    ns, fn = filler.popleft()
                    fn()
                    budget -= ns

            def pump_all():
                while filler:
                    filler.popleft()[1]()

            def add_closures(lst):
                """Queue a chunk's closures, splicing any pending tail
                (previous chunk's trailing transposes) after the first
                few so the PE has ready matmuls while rotary runs."""
                nonlocal pending
                if pending:
                    lst = lst[:3] + pending + lst[3:]
                    pending = []
                filler.extend(lst)

            def flush_pending():
                nonlocal pending
                filler.extend(pending)
                pending = []

            def nonlocal_pending(tl):
                nonlocal pending
                pending = pending + tl

            # ----------------- closure builders -----------------
            def add_xdma(b):
                # b0's x loads go out mostly on the scalar-engine HWDGE
                # queue (idle during the prologue) so they transfer in
                # parallel with the weight loads on sync; the last quarter
                # rides sync to balance the two queues.
                def f():
                    t = sb.tile([128, ND * N], BF16, name=f"xT_b{b}",
                                tag="xT")
                    dst = t[:].rearrange("p (d n) -> p d n", n=N)
                    src = xT_d.ap()[b].rearrange("(d p) n -> p d n", p=128)
                    for u in range(4):
                        eng = (nc.scalar if (b == 0 and u < 3) else
                               nc.sync)
                        eng.dma_start(dst[:, 2 * u:2 * u + 2, :],
                                      src[:, 2 * u:2 * u + 2, :])
                    xTt[b] = t
                filler.append((60, f))

            def build_chunk(b, j0, tag, consume_final, grp_hook=None,
                            mark=None, mid_hook=None):
                """Build closure list for qkv[:, j0:j0+512] per token tile.

                Returns (closures, tail): trailing hook closures of the
                last group go into `tail` so the caller can splice them
                after the next chunk's first matmuls (PE keeps streaming
                while the rotary DVE chain runs).
                grp_hook(grp) -> list of (ns, fn) hook closures.
                """
                wq_box = {}

                def loadw():
                    w = sb.tile([128, ND * 512], BF16, name=f"wq_{tag}",
                                tag="wq", bufs=3)
                    dst = w[:].rearrange("p (d c) -> p d c", c=512)
                    src = wqkvT_d.ap()[:, j0:j0 + 512].rearrange(
                        "(d p) c -> p d c", p=128)
                    nc.sync.dma_start(dst[:, 0:4, :], src[:, 0:4, :])
                    nc.sync.dma_start(dst[:, 4:8, :], src[:, 4:8, :])
                    wq_box[0] = w

                pq_box = {}

                def mk(t, q):
                    def f():
                        if q == 0:
                            pq_box[t] = ps.tile([128, 512], F32,
                                                name=f"pq_{tag}_{t}",
                                                tag="mm512", bufs=2)
                        pq = pq_box[t]
                        xb, wq = xTt[b], wq_box[0]
                        for d in (2 * q, 2 * q + 1):
                            nc.tensor.matmul(
                                pq[:],
                                xb[:, d * N + t * 128:d * N + (t + 1) * 128],
                                wq[:, d * 512:(d + 1) * 512],
                                start=(d == 0), stop=(d == ND - 1))
                        if q == 3:
                            consume_final(t, pq)
                            del pq_box[t]
                    return f

                out = [(60, loadw)]
                carry = []   # hooks of group 0, spliced into group 1 MMs
                tail = []
                for t in range(NT):
                    for q in range(4):
                        out.append((540, mk(t, q)))
                        if carry and q % 2 == 1:
                            out.append(carry.pop(0))
                    if mid_hook is not None and t % 4 == 1:
                        out.extend(mid_hook(t // 4))
                    if grp_hook is not None and t % 4 == 3:
                        hooks = grp_hook(t // 4)
                        if t // 4 == 0:
                            # rotary first (DVE, free), transposes carried
                            out.append(hooks[0])
                            carry = hooks[1:]
                        else:
                            out.append(hooks[0])
                            tail = hooks[1:]
                out.extend(carry)
                if mark is not None:
                    tail = tail + [(0, lambda: done.add(mark))]
                return out, tail

            def add_v_chunk(b, jc):
                """V chunk jc: heads 8jc..8jc+7 -> vsL/vsR tiles."""
                side = vsL if jc == 0 else vsR

                def eat(t, pq, jc=jc, side=side):
                    if (b, t) not in side:
                        vs = sb.tile([128, 8 * (HD + 1)], BF16,
                                     name=f"v{'LR'[jc]}_b{b}_{t}",
                                     tag=f"vs{'LR'[jc]}{t}")
                        nc.vector.memset(
                            vs[:].rearrange("p (h c) -> p h c",
                                            c=HD + 1)[:, :, HD:], 1.0)
                        side[(b, t)] = vs
                    vs = side[(b, t)]
                    nc.vector.tensor_copy(
                        vs[:].rearrange("p (h c) -> p h c",
                                        c=HD + 1)[:, :, 0:HD],
                        pq[:].rearrange("p (h c) -> p h c", c=HD))
                cl, tl = build_chunk(b, 2 * DIM + jc * 512, f"v{b}{jc}",
                                     eat, mark=("v", b, jc))
                add_closures(cl)
                nonlocal_pending(tl)

            def add_kq_chunk(b, sect, g, dst):
                """K (sect=1) or Q (sect=0) chunk g: heads 8g..8g+7 ->
                rotary + transpose into dst[(b, 4g+jt)] tiles."""
                tag = f"s{sect}b{b}g{g}"
                qnw_box = {}

                def eat(t, pq):
                    grp, tl = t // 4, t % 4
                    if tl == 0:
                        qnw_box[grp] = sb.tile([128, 2048], BF16,
                                               name=f"qnw_{tag}_{grp}",
                                               tag="qnw", bufs=2)
                    nc.vector.tensor_copy(
                        qnw_box[grp][:, tl * 512:(tl + 1) * 512], pq[:])

                def rot_half(grp, lo):
                    """Rotary on token tiles grp*4+lo, grp*4+lo+1."""
                    def rotary():
                        qnw = qnw_box[grp]
                        v4 = qnw[:].rearrange("p (t h c) -> p t h c",
                                              t=4, c=HD)
                        ev = v4[:, lo:lo + 2, :, 0:16]
                        od = v4[:, lo:lo + 2, :, 16:32]
                        cs = _ins_bcast(
                            cosh[:].rearrange("p (t c) -> p t c", c=16)
                            [:, grp * 4 + lo:grp * 4 + lo + 2, :], 8, 1)
                        sn = _ins_bcast(
                            sinh[:].rearrange("p (t c) -> p t c", c=16)
                            [:, grp * 4 + lo:grp * 4 + lo + 2, :], 8, 1)
                        ts4 = [sb.tile([128, 2, 8, 16], BF16,
                                       name=f"rt{i}_{tag}_{grp}_{lo}",
                                       tag=f"rt{i}", bufs=2)
                               for i in range(4)]
                        nc.vector.tensor_tensor(ts4[0][:], ev, cs, mul)
                        nc.vector.tensor_tensor(ts4[1][:], od, sn, mul)
                        nc.vector.tensor_tensor(ts4[2][:], od, cs, mul)
                        nc.vector.tensor_tensor(ts4[3][:], ev, sn, mul)
                        nc.vector.tensor_tensor(ev, ts4[0][:], ts4[1][:], sub)
                        nc.vector.tensor_tensor(od, ts4[2][:], ts4[3][:], add)
                    return rotary

                def mid_hook(grp):
                    return [(0, rot_half(grp, 0))]

                def grp_hook(grp):
                    def mk(jt):
                        def f():
                            j = 4 * g + jt
                            if (b, j) not in dst:
                                dst[(b, j)] = sb.tile(
                                    [128, N], BF16,
                                    name=f"{'qk'[sect]}T_b{b}_{j}",
                                    tag=f"{'qk'[sect]}T{j}", bufs=2)
                            qnw = qnw_box[grp]
                            tp = ps.tile([128, 512], BF16,
                                         name=f"tp_{tag}_{grp}_{jt}",
                                         tag="mm512", bufs=2)
                            for u in range(4):
                                nc.tensor.transpose(
                                    tp[:, u * 128:(u + 1) * 128],
                                    qnw[:, u * 512 + jt * 128:
                                        u * 512 + jt * 128 + 128],
                                    ident[:])
                            nc.vector.tensor_copy(
                                dst[(b, j)][:, grp * 512:(grp + 1) * 512],
                                tp[:])
                        return f
                    return ([(0, rot_half(grp, 2))] +
                            [(660, mk(jt)) for jt in range(4)])

                cl, tl = build_chunk(b, sect * DIM + g * 512, tag, eat,
                                     grp_hook,
                                     mark=("qk"[sect], b, g),
                                     mid_hook=mid_hook)
                add_closures(cl)
                nonlocal_pending(tl)

            def add_proj(b, ecs=(0, 1), wbig=False):
                wp_box = {}

                def loadw_big():
                    def f():
                        w = sb.tile([128, ND * DIM], BF16,
                                    name=f"wpbig_b{b}", tag="xT")
                        nc.sync.dma_start(
                            w[:].rearrange("p (d c) -> p d c", c=DIM),
                            wprojT_d.ap().rearrange("(d p) c -> p d c",
                                                    p=128))
                        wp_box["big"] = w
                    return f

                def loadw(ec):
                    def f():
                        wp_box.clear()
                        for d in range(ND):
                            w = sb.tile([128, 512], BF16,
                                        name=f"wp_b{b}_{ec}_{d}",
                                        tag=f"wp{d}")
                            nc.sync.dma_start(
                                w[:], wprojT_d.ap()[d * 128:(d + 1) * 128,
                                                    ec * 512:(ec + 1) * 512])
                            wp_box[d] = w
                    return f

                py_box = {}

                def mk(ec, t, q):
                    def f():
                        if q == 0:
                            py_box[t] = ps.tile([128, 512], F32,
                                                name=f"py_b{b}_{t}_{ec}",
                                                tag="mm512", bufs=2)
                        py = py_box[t]
                        for d in (2 * q, 2 * q + 1):
                            if wbig:
                                wmov = wp_box["big"][
                                    :, d * DIM + ec * 512:
                                    d * DIM + ec * 512 + 512]
                            else:
                                wmov = wp_box[d][:]
                            nc.tensor.matmul(
                                py[:], outT[(b, d)][:, t * 128:(t + 1) * 128],
                                wmov, start=(d == 0),
                                stop=(d == ND - 1))
                        if q == 3:
                            ysb = sb.tile([128, 512], F32,
                                          name=f"y_b{b}_{t}_{ec}",
                                          tag="ysb", bufs=2)
                            nc.vector.tensor_tensor(
                                ysb[:], py[:],
                                bias_bc[:, ec * 512:(ec + 1) * 512], add)
                            nc.sync.dma_start(
                                y_d.ap()[b, t * 128:(t + 1) * 128,
                                         ec * 512:(ec + 1) * 512], ysb[:])
                            del py_box[t]
                    return f
                if wbig:
                    filler.append((60, loadw_big()))
                for ec in ecs:
                    if not wbig:
                        filler.append((60, loadw(ec)))
                    for t in range(NT):
                        for q in range(4):
                            filler.append((540, mk(ec, t, q)))

            # ----------------- attention -----------------
            def attention(b, hp):
                # force-pump until dependency chunks are fully emitted
                g = hp // 4
                need = lambda: (("k", b, g) in done and ("q", b, g) in done
                                and ("v", b, hp // 4) in done)
                while not need():
                    if not filler:
                        flush_pending()
                        if not filler:
                            raise RuntimeError(f"filler empty, deps missing "
                                               f"b{b} hp{hp}")
                    pump(2000)
                kk, qq = kT[(b, hp)], qT[(b, hp)]
                side = vsL if hp < 4 else vsR
                cA = ((2 * hp) % 8) * (HD + 1)
                cB = cA + (HD + 1)
                if (b, hp) not in outT:
                    outT[(b, hp)] = sb.tile([128, N], BF16,
                                            name=f"outT_b{b}_{hp}",
                                            tag=f"outT{hp}", bufs=2)
                for nch in range(2):
                    pvA = ps.tile([HD + 1, 512], F32, name=f"pvA_b{b}_{hp}_{nch}",
                                  tag="pvA", bufs=1)
                    pvB = ps.tile([HD + 1, 512], F32, name=f"pvB_b{b}_{hp}_{nch}",
                                  tag="pvB", bufs=1)

                    def emit_pv(m, ptp):
                        nc.tensor.matmul(
                            pvA[:], side[(b, m)][:, cA:cA + HD + 1],
                            ptp[:, 0:512], start=(m == 0), stop=(m == NT - 1))
                        nc.tensor.matmul(
                            pvB[:], side[(b, m)][:, cB:cB + HD + 1],
                            ptp[:, 512:1024], start=(m == 0),
                            stop=(m == NT - 1))

                    prev = None
                    for m in range(NT):
                        stp = ps.tile([128, 1024], F32,
                                      name=f"st_b{b}_{hp}_{m}_{nch}",
                                      tag="st", bufs=2)
                        nc.tensor.matmul(
                            stp[:, 0:512], kk[0:64, m * 128:(m + 1) * 128],
                            qq[0:64, nch * 512:(nch + 1) * 512])
                        nc.tensor.matmul(
                            stp[:, 512:1024], kk[64:128, m * 128:(m + 1) * 128],
                            qq[64:128, nch * 512:(nch + 1) * 512])
                        ptp = sb.tile([128, 1024], BF16,
                                      name=f"pt_b{b}_{hp}_{m}_{nch}",
                                      tag="ptp", bufs=2)
                        nc.scalar.activation(ptp[:], stp[:], EXP,
                                             scale=float(SCALE))
                        if prev is not None:
                            emit_pv(*prev)
                        prev = (m, ptp)
                        pump(650)
                    emit_pv(*prev)

                    # evacuate + normalize
                    for tagc, pv, r0 in (("A", pvA, 0), ("B", pvB, 64)):
                        pvs = sb.tile([HD + 1, 512], F32,
                                      name=f"pvs{tagc}_b{b}_{hp}_{nch}",
                                      tag=f"pvs{tagc}")
                        nc.scalar.activation(pvs[:], pv[:], CPY)
                        dr = sb.tile([1, 512], F32,
                                     name=f"dr{tagc}_b{b}_{hp}_{nch}",
                                     tag=f"dr{tagc}")
                        nc.vector.tensor_copy(dr[:], pvs[64:65, :])
                        rr = sb.tile([1, 512], F32,
                                     name=f"rr{tagc}_b{b}_{hp}_{nch}",
                                     tag=f"rr{tagc}")
                        nc.vector.reciprocal_approx_fast(rr[:], dr[:])
                        rb = sb.tile([64, 512], F32,
                                     name=f"rb{tagc}_b{b}_{hp}_{nch}",
                                     tag=f"rb{tagc}")
                        nc.gpsimd.partition_broadcast(rb[:], rr[:])
                        nc.vector.tensor_tensor(
                            outT[(b, hp)][r0:r0 + 64,
                                          nch * 512:(nch + 1) * 512],
                            pvs[0:64, :], rb[:], mul)
                    pump(600)

            # ----------------- main schedule -----------------
            add_xdma(0)
            add_v_chunk(0, 0)
            pump(360)        # issue x/w DMAs ahead of the const DMAs
            emit_consts()
            add_kq_chunk(0, 1, 0, kT)
            add_kq_chunk(0, 0, 0, qT)
            pump_all()

            add_v_chunk(0, 1)
            add_kq_chunk(0, 1, 1, kT)
            add_kq_chunk(0, 0, 1, qT)
            for hp in range(NHP):
                if hp == 3:
                    add_xdma(1)
                    add_kq_chunk(1, 1, 0, kT)
                elif hp == 6:
                    add_kq_chunk(1, 0, 0, qT)
                attention(0, hp)
            flush_pending()
            pump_all()

            add_v_chunk(1, 0)
            flush_pending()
            pump_all()

            add_v_chunk(1, 1)
            add_kq_chunk(1, 1, 1, kT)
            add_kq_chunk(1, 0, 1, qT)
            for hp in range(NHP):
                if hp == 3:
                    add_proj(0, (0,))
                elif hp == 5:
                    add_proj(0, (1,))
                attention(1, hp)
            flush_pending()
            pump_all()

            add_proj(1, wbig=True)
            pump_all()

    nc.compile()
    return nc


_NC_CACHE = None


def kernel(x, w_qkv, w_proj, b_proj):
    global _NC_CACHE, last_exec_time_ns
    import ml_dtypes

    x = np.ascontiguousarray(np.asarray(x, np.float32))
    w_qkv = np.asarray(w_qkv, np.float32)
    w_proj = np.asarray(w_proj, np.float32)
    b_proj = np.asarray(b_proj, np.float32)

    if _NC_CACHE is None:
        _NC_CACHE = _build()
    nc = _NC_CACHE

    cos_h, sin_h = _freq_tables()
    perm = _qk_perm()
    wq_p = np.empty_like(w_qkv)
    wq_p[0:DIM] = w_qkv[0:DIM][perm]            # q rows permuted
    wq_p[DIM:2 * DIM] = w_qkv[DIM:2 * DIM][perm]  # k rows permuted
    wq_p[2 * DIM:] = w_qkv[2 * DIM:]            # v rows unchanged
    wqkvT = np.ascontiguousarray(wq_p.T).astype(ml_dtypes.bfloat16)
    wprojT = np.ascontiguousarray(w_proj.T).astype(ml_dtypes.bfloat16)
    bproj16 = b_proj.reshape(1, DIM).astype(ml_dtypes.bfloat16)
    # pre-arrange to the [128, NT*16] sbuf layout: [p, t*16+c] = table[t*128+p, c]
    cos16 = np.ascontiguousarray(
        cos_h.reshape(NT, 128, 16).transpose(1, 0, 2).reshape(128, NT * 16)
    ).astype(ml_dtypes.bfloat16)
    sin16 = np.ascontiguousarray(
        sin_h.reshape(NT, 128, 16).transpose(1, 0, 2).reshape(128, NT * 16)
    ).astype(ml_dtypes.bfloat16)
    ident = np.eye(128, dtype=np.float32).astype(ml_dtypes.bfloat16)

    in_maps = []
    for c in range(NCORES):
        xs = x[c * BPC:(c + 1) * BPC]                       # [2, N, DIM]
        xT = np.ascontiguousarray(xs.transpose(0, 2, 1)).astype(
            ml_dtypes.bfloat16)
        in_maps.append({
            "xT": xT, "wqkvT": wqkvT, "wprojT": wprojT,
            "bproj": bproj16, "cosh": cos16, "sinh": sin16,
            "ident": ident,
        })

    trace = bool(os.environ.get("KERNEL_TRACE"))
    kwargs = {}
    if trace:
        kwargs["trace"] = True
        td = os.environ.get("KERNEL_TRACE_DIR")
        if td:
            kwargs["tmpdir"] = td
    res = bass_utils.run_bass_kernel_spmd(
        nc, in_maps, core_ids=list(range(NCORES)), **kwargs)
    last_exec_time_ns = res.exec_time_ns
    out = np.concatenate([res.results[c]["y"] for c in range(NCORES)], axis=0)
    return np.ascontiguousarray(out.reshape(B, N, DIM).astype(np.float32))


if __name__ == "__main__":
    rng = np.random.default_rng(0)
    xs = rng.standard_normal((B, N, DIM), dtype=np.float32)
    wq = rng.standard_normal((3 * DIM, DIM), dtype=np.float32) / 32
    wp = rng.standard_normal((DIM, DIM), dtype=np.float32) / 32
    bp = np.zeros(DIM, np.float32)
    y = kernel(xs, wq, wp, bp)
    print("y", y.shape, y.dtype, float(np.abs(y).max()))


# revision 43
# speedup vs baseline: 1.0033x; 1.0033x over previous
"""Trainium2 Bass kernel for nn_Attention_13348758356565.

Dense attention block (B=16, N=1024, DIM=1024, 16 heads x 64) with axial
rotary embeddings, data-parallel over batch across 8 NeuronCores (2 batches
per core). ~543us HW vs ~1288us f32r baseline (2.4x), rel_absmax ~8.5e-3.

Design notes (hardware-measured, CoreSim cost model diverges):
- All matmul operands bf16: HW streams fp32/f32r moving operands at ~2
  cyc/col (4B bandwidth limit) but bf16 at 1 cyc/col (~216ns per 512-col
  MM warm) => 2x PE throughput vs the f32r baseline.
- QK^T packed as 2-head row-group pairs (stationary at base partitions
  0/64, K=64): the two MMs run concurrently on the PE array (4ns apart).
- One exp ACTIVATE per [128,1024] score pair spanning 2 PSUM banks
  (matmul dsts stay per-bank; ACT reads may cross banks) -> ~1025ns vs
  2x824ns split.
- Softmax denominators ride a 65th ones-column in the PV stationary (any
  separate denominator pass costs exactly the stream time it would save).
- PV psum evacuation on the scalar engine (ACT Copy); reciprocal of the
  denominator row must first be copied to partition 0 (HW's
  reciprocal_approx_fast ignores a nonzero base partition - sim doesn't).
- Emission-interleaved "filler": QKV/rotary/transpose/out-proj work of the
  other batch is pumped between attention iterations so the PE never idles
  while scalar runs exp (keeps HAM at K=8/8). Trailing per-group transposes
  are carried into the next chunk's matmul stream because the TileScheduler
  hands out PSUM slots in priority order (a transpose waiting on the rotary
  DVE chain would otherwise block the next psum chain).
- Rotary on natural-layout bf16 tiles with an [evens|odds|pass] head-dim
  permutation folded into w_qkv rows host-side (packed last-dim slices ->
  DVE 2x/4x modes), emitted in two halves so its latency hides under the
  psum-evacuation CASTs; Q/K transposed to [dim, token] via PE transposes.
"""

import os
import sys

sys.path.insert(0, "/opt/trn_rl_repo")

import dataclasses
from collections import deque

import numpy as np

import concourse.bacc as bacc
import concourse.mybir as mybir
import concourse.tile as tile
from concourse import bass_utils

F32 = mybir.dt.float32
BF16 = mybir.dt.bfloat16
EXP = mybir.ActivationFunctionType.Exp
CPY = mybir.ActivationFunctionType.Copy

B, HF, WF = 16, 32, 32
DIM, NH, HD = 1024, 16, 64
N = HF * WF          # 1024 tokens
NCORES = 8
BPC = B // NCORES    # 2 batches per core
ROT = HD // 2        # 32 rotary dims per head
NT = N // 128        # 8 token tiles
ND = DIM // 128      # 8 contraction tiles
NHP = NH // 2        # 8 head pairs
SCALE = 1.0 / np.sqrt(HD)

last_exec_time_ns = None


def _freq_tables():
    d = HD // 4
    base = (np.linspace(1.0, (HF * WF) / 2.0, d // 2, dtype=np.float64) * np.pi)
    posH = np.linspace(-1.0, 1.0, HF)
    posW = np.linspace(-1.0, 1.0, WF)
    fH = np.repeat(posH[:, None] * base[None, :], 2, axis=-1)   # [H, 16]
    fW = np.repeat(posW[:, None] * base[None, :], 2, axis=-1)   # [W, 16]
    fH = np.broadcast_to(fH[:, None, :], (HF, WF, d))
    fW = np.broadcast_to(fW[None, :, :], (HF, WF, d))
    freqs = np.concatenate([fH, fW], axis=-1).reshape(N, ROT)
    # freqs[:, 2i] == freqs[:, 2i+1]; keep one per pair -> [N, 16]
    half = freqs[:, 0::2]
    return np.cos(half), np.sin(half)


def _qk_perm():
    """Per-head dim order [e0..e15, o0..o15, pass0..pass31]."""
    per_head = np.concatenate([
        np.arange(0, ROT, 2), np.arange(1, ROT, 2), np.arange(ROT, HD)])
    return (np.arange(NH)[:, None] * HD + per_head[None, :]).reshape(-1)


def _ins_bcast(ap, count, pos):
    """Insert a step-0 dim of size `count` at free-dim position `pos`."""
    dims = list(ap.ap)
    dims.insert(pos + 1, [0, count])
    return dataclasses.replace(ap, ap=dims)


def _build():
    nc = bacc.Bacc("TRN2", target_bir_lowering=False, debug=False)

    xT_d = nc.dram_tensor("xT", [BPC, DIM, N], BF16, kind="ExternalInput")
    wqkvT_d = nc.dram_tensor("wqkvT", [DIM, 3 * DIM], BF16, kind="ExternalInput")
    wprojT_d = nc.dram_tensor("wprojT", [DIM, DIM], BF16, kind="ExternalInput")
    bproj_d = nc.dram_tensor("bproj", [1, DIM], BF16, kind="ExternalInput")
    cosh_d = nc.dram_tensor("cosh", [128, NT * 16], BF16,
                            kind="ExternalInput")
    sinh_d = nc.dram_tensor("sinh", [128, NT * 16], BF16,
                            kind="ExternalInput")
    ident_d = nc.dram_tensor("ident", [128, 128], BF16, kind="ExternalInput")
    y_d = nc.dram_tensor("y", [BPC, N, DIM], F32, kind="ExternalOutput")

    mul = mybir.AluOpType.mult
    sub = mybir.AluOpType.subtract
    add = mybir.AluOpType.add

    with tile.TileContext(nc) as tc:
        with (
            tc.tile_pool(name="sb", bufs=1) as sb,
            tc.tile_pool(name="ps", bufs=1, space="PSUM") as ps,
        ):
            # ---- constants (emitted after the first input DMAs) ----
            cosh = sinh = ident = bias_bc = None

            def emit_consts():
                nonlocal cosh, sinh, ident, bias_bc
                ident = sb.tile([128, 128], BF16, name="ident")
                nc.scalar.dma_start(ident[:], ident_d.ap())
                cosh = sb.tile([128, NT * 16], BF16, name="cosh")
                sinh = sb.tile([128, NT * 16], BF16, name="sinh")
                nc.scalar.dma_start(cosh[:], cosh_d.ap())
                nc.scalar.dma_start(sinh[:], sinh_d.ap())
                bias_row = sb.tile([1, DIM], BF16, name="bias_row")
                nc.scalar.dma_start(bias_row[:], bproj_d.ap())
                bias_bc = sb.tile([128, DIM], BF16, name="bias_bc")
                nc.gpsimd.partition_broadcast(bias_bc[:], bias_row[:])

            # live tile registries (filled by closures at emission time)
            xTt, kT, qT, vsL, vsR, outT = {}, {}, {}, {}, {}, {}
            done = set()   # emission-complete markers for chunks

            filler = deque()
            pending = []   # tail closures to splice into the next chunk

            def pump(budget):
                while budget > 0 and filler:
                    ns, fn = filler.popleft()
                    fn()
                    budget -= ns

            def pump_all():
                while filler:
                    filler.popleft()[1]()

            def add_closures(lst):
                """Queue a chunk's closures, splicing any pending tail
                (previous chunk's trailing transposes) after the first
                few so the PE has ready matmuls while rotary runs."""
                nonlocal pending
                if pending:
                    lst = lst[:3] + pending + lst[3:]
                    pending = []
                filler.extend(lst)

            def flush_pending():
                nonlocal pending
                filler.extend(pending)
                pending = []

            def nonlocal_pending(tl):
                nonlocal pending
                pending = pending + tl

            # ----------------- closure builders -----------------
            def add_xdma(b):
                # b0's x loads go out on the scalar-engine HWDGE queue so
                # they transfer in parallel with the weight loads on sync
                # (scalar is idle during the prologue).
                eng = nc.scalar if b == 0 else nc.sync

                def f():
                    t = sb.tile([128, ND * N], BF16, name=f"xT_b{b}",
                                tag="xT")
                    dst = t[:].rearrange("p (d n) -> p d n", n=N)
                    src = xT_d.ap()[b].rearrange("(d p) n -> p d n", p=128)
                    for u in range(4):
                        eng.dma_start(dst[:, 2 * u:2 * u + 2, :],
                                      src[:, 2 * u:2 * u + 2, :])
                    xTt[b] = t
                filler.append((60, f))

            def build_chunk(b, j0, tag, consume_final, grp_hook=None,
                            mark=None, mid_hook=None):
                """Build closure list for qkv[:, j0:j0+512] per token tile.

                Returns (closures, tail): trailing hook closures of the
                last group go into `tail` so the caller can splice them
                after the next chunk's first matmuls (PE keeps streaming
                while the rotary DVE chain runs).
                grp_hook(grp) -> list of (ns, fn) hook closures.
                """
                wq_box = {}

                def loadw():
                    w = sb.tile([128, ND * 512], BF16, name=f"wq_{tag}",
                                tag="wq", bufs=3)
                    dst = w[:].rearrange("p (d c) -> p d c", c=512)
                    src = wqkvT_d.ap()[:, j0:j0 + 512].rearrange(
                        "(d p) c -> p d c", p=128)
                    nc.sync.dma_start(dst[:, 0:4, :], src[:, 0:4, :])
                    nc.sync.dma_start(dst[:, 4:8, :], src[:, 4:8, :])
                    wq_box[0] = w

                pq_box = {}

                def mk(t, q):
                    def f():
                        if q == 0:
                            pq_box[t] = ps.tile([128, 512], F32,
                                                name=f"pq_{tag}_{t}",
                                                tag="mm512", bufs=2)
                        pq = pq_box[t]
                        xb, wq = xTt[b], wq_box[0]
                        for d in (2 * q, 2 * q + 1):
                            nc.tensor.matmul(
                                pq[:],
                                xb[:, d * N + t * 128:d * N + (t + 1) * 128],
                                wq[:, d * 512:(d + 1) * 512],
                                start=(d == 0), stop=(d == ND - 1))
                        if q == 3:
                            consume_final(t, pq)
                            del pq_box[t]
                    return f

                out = [(60, loadw)]
                carry = []   # hooks of group 0, spliced into group 1 MMs
                tail = []
                for t in range(NT):
                    for q in range(4):
                        out.append((540, mk(t, q)))
                        if carry and q % 2 == 1:
                            out.append(carry.pop(0))
                    if mid_hook is not None and t % 4 == 1:
                        out.extend(mid_hook(t // 4))
                    if grp_hook is not None and t % 4 == 3:
                        hooks = grp_hook(t // 4)
                        if t // 4 == 0:
                            # rotary first (DVE, free), transposes carried
                            out.append(hooks[0])
                            carry = hooks[1:]
                        else:
                            out.append(hooks[0])
                            tail = hooks[1:]
                out.extend(carry)
                if mark is not None:
                    tail = tail + [(0, lambda: done.add(mark))]
                return out, tail

            def add_v_chunk(b, jc):
                """V chunk jc: heads 8jc..8jc+7 -> vsL/vsR tiles."""
                side = vsL if jc == 0 else vsR

                def eat(t, pq, jc=jc, side=side):
                    if (b, t) not in side:
                        vs = sb.tile([128, 8 * (HD + 1)], BF16,
                                     name=f"v{'LR'[jc]}_b{b}_{t}",
                                     tag=f"vs{'LR'[jc]}{t}")
                        nc.vector.memset(
                            vs[:].rearrange("p (h c) -> p h c",
                                            c=HD + 1)[:, :, HD:], 1.0)
                        side[(b, t)] = vs
                    vs = side[(b, t)]
                    nc.vector.tensor_copy(
                        vs[:].rearrange("p (h c) -> p h c",
                                        c=HD + 1)[:, :, 0:HD],
                        pq[:].rearrange("p (h c) -> p h c", c=HD))
                cl, tl = build_chunk(b, 2 * DIM + jc * 512, f"v{b}{jc}",
                                     eat, mark=("v", b, jc))
                add_closures(cl)
                nonlocal_pending(tl)

            def add_kq_chunk(b, sect, g, dst):
                """K (sect=1) or Q (sect=0) chunk g: heads 8g..8g+7 ->
                rotary + transpose into dst[(b, 4g+jt)] tiles."""
                tag = f"s{sect}b{b}g{g}"
                qnw_box = {}

                def eat(t, pq):
                    grp, tl = t // 4, t % 4
                    if tl == 0:
                        qnw_box[grp] = sb.tile([128, 2048], BF16,
                                               name=f"qnw_{tag}_{grp}",
                                               tag="qnw", bufs=2)
                    nc.vector.tensor_copy(
                        qnw_box[grp][:, tl * 512:(tl + 1) * 512], pq[:])

                def rot_half(grp, lo):
                    """Rotary on token tiles grp*4+lo, grp*4+lo+1."""
                    def rotary():
                        qnw = qnw_box[grp]
                        v4 = qnw[:].rearrange("p (t h c) -> p t h c",
                                              t=4, c=HD)
                        ev = v4[:, lo:lo + 2, :, 0:16]
                        od = v4[:, lo:lo + 2, :, 16:32]
                        cs = _ins_bcast(
                            cosh[:].rearrange("p (t c) -> p t c", c=16)
                            [:, grp * 4 + lo:grp * 4 + lo + 2, :], 8, 1)
                        sn = _ins_bcast(
                            sinh[:].rearrange("p (t c) -> p t c", c=16)
                            [:, grp * 4 + lo:grp * 4 + lo + 2, :], 8, 1)
                        ts4 = [sb.tile([128, 2, 8, 16], BF16,
                                       name=f"rt{i}_{tag}_{grp}_{lo}",
                                       tag=f"rt{i}", bufs=2)
                               for i in range(4)]
                        nc.vector.tensor_tensor(ts4[0][:], ev, cs, mul)
                        nc.vector.tensor_tensor(ts4[1][:], od, sn, mul)
                        nc.vector.tensor_tensor(ts4[2][:], od, cs, mul)
                        nc.vector.tensor_tensor(ts4[3][:], ev, sn, mul)
                        nc.vector.tensor_tensor(ev, ts4[0][:], ts4[1][:], sub)
                        nc.vector.tensor_tensor(od, ts4[2][:], ts4[3][:], add)
                    return rotary

                def mid_hook(grp):
                    return [(0, rot_half(grp, 0))]

                def grp_hook(grp):
                    def mk(jt):
                        def f():
                            j = 4 * g + jt
                            if (b, j) not in dst:
                                dst[(b, j)] = sb.tile(
                                    [128, N], BF16,
                                    name=f"{'qk'[sect]}T_b{b}_{j}",
                                    tag=f"{'qk'[sect]}T{j}", bufs=2)
                            qnw = qnw_box[grp]
                            tp = ps.tile([128, 512], BF16,
                                         name=f"tp_{tag}_{grp}_{jt}",
                                         tag="mm512", bufs=2)
                            for u in range(4):
                                nc.tensor.transpose(
                                    tp[:, u * 128:(u + 1) * 128],
                                    qnw[:, u * 512 + jt * 128:
                                        u * 512 + jt * 128 + 128],
                                    ident[:])
                            nc.vector.tensor_copy(
                                dst[(b, j)][:, grp * 512:(grp + 1) * 512],
                                tp[:])
                        return f
                    return ([(0, rot_half(grp, 2))] +
                            [(660, mk(jt)) for jt in range(4)])

                cl, tl = build_chunk(b, sect * DIM + g * 512, tag, eat,
                                     grp_hook,
                                     mark=("qk"[sect], b, g),
                                     mid_hook=mid_hook)
                add_closures(cl)
                nonlocal_pending(tl)

            def add_proj(b, ecs=(0, 1), wbig=False):
                wp_box = {}

                def loadw_big():
                    def f():
                        w = sb.tile([128, ND * DIM], BF16,
                                    name=f"wpbig_b{b}", tag="xT")
                        nc.sync.dma_start(
                            w[:].rearrange("p (d c) -> p d c", c=DIM),
                            wprojT_d.ap().rearrange("(d p) c -> p d c",
                                                    p=128))
                        wp_box["big"] = w
                    return f

                def loadw(ec):
                    def f():
                        wp_box.clear()
                        for d in range(ND):
                            w = sb.tile([128, 512], BF16,
                                        name=f"wp_b{b}_{ec}_{d}",
                                        tag=f"wp{d}")
                            nc.sync.dma_start(
                                w[:], wprojT_d.ap()[d * 128:(d + 1) * 128,
                                                    ec * 512:(ec + 1) * 512])
                            wp_box[d] = w
                    return f

                py_box = {}

                def mk(ec, t, q):
                    def f():
                        if q == 0:
                            py_box[t] = ps.tile([128, 512], F32,
                                                name=f"py_b{b}_{t}_{ec}",
                                                tag="mm512", bufs=2)
                        py = py_box[t]
                        for d in (2 * q, 2 * q + 1):
                            if wbig:
                                wmov = wp_box["big"][
                                    :, d * DIM + ec * 512:
                                    d * DIM + ec * 512 + 512]
                            else:
                                wmov = wp_box[d][:]
                            nc.tensor.matmul(
                                py[:], outT[(b, d)][:, t * 128:(t + 1) * 128],
                                wmov, start=(d == 0),
                                stop=(d == ND - 1))
                        if q == 3:
                            ysb = sb.tile([128, 512], F32,
                                          name=f"y_b{b}_{t}_{ec}",
                                          tag="ysb", bufs=2)
                            nc.vector.tensor_tensor(
                                ysb[:], py[:],
                                bias_bc[:, ec * 512:(ec + 1) * 512], add)
                            nc.sync.dma_start(
                                y_d.ap()[b, t * 128:(t + 1) * 128,
                                         ec * 512:(ec + 1) * 512], ysb[:])
                            del py_box[t]
                    return f
                if wbig:
                    filler.append((60, loadw_big()))
                for ec in ecs:
                    if not wbig:
                        filler.append((60, loadw(ec)))
                    for t in range(NT):
                        for q in range(4):
                            filler.append((540, mk(ec, t, q)))

            # ----------------- attention -----------------
            def attention(b, hp):
                # force-pump until dependency chunks are fully emitted
                g = hp // 4
                need = lambda: (("k", b, g) in done and ("q", b, g) in done
                                and ("v", b, hp // 4) in done)
                while not need():
                    if not filler:
                        flush_pending()
                        if not filler:
                            raise RuntimeError(f"filler empty, deps missing "
                                               f"b{b} hp{hp}")
                    pump(2000)
                kk, qq = kT[(b, hp)], qT[(b, hp)]
                side = vsL if hp < 4 else vsR
                cA = ((2 * hp) % 8) * (HD + 1)
                cB = cA + (HD + 1)
                if (b, hp) not in outT:
                    outT[(b, hp)] = sb.tile([128, N], BF16,
                                            name=f"outT_b{b}_{hp}",
                                            tag=f"outT{hp}", bufs=2)
                for nch in range(2):
                    pvA = ps.tile([HD + 1, 512], F32, name=f"pvA_b{b}_{hp}_{nch}",
                                  tag="pvA", bufs=1)
                    pvB = ps.tile([HD + 1, 512], F32, name=f"pvB_b{b}_{hp}_{nch}",
                                  tag="pvB", bufs=1)

                    def emit_pv(m, ptp):
                        nc.tensor.matmul(
                            pvA[:], side[(b, m)][:, cA:cA + HD + 1],
                            ptp[:, 0:512], start=(m == 0), stop=(m == NT - 1))
                        nc.tensor.matmul(
                            pvB[:], side[(b, m)][:, cB:cB + HD + 1],
                            ptp[:, 512:1024], start=(m == 0),
                            stop=(m == NT - 1))

                    prev = None
                    for m in range(NT):
                        stp = ps.tile([128, 1024], F32,
                                      name=f"st_b{b}_{hp}_{m}_{nch}",
                                      tag="st", bufs=2)
                        nc.tensor.matmul(
                            stp[:, 0:512], kk[0:64, m * 128:(m + 1) * 128],
                            qq[0:64, nch * 512:(nch + 1) * 512])
                        nc.tensor.matmul(
                            stp[:, 512:1024], kk[64:128, m * 128:(m + 1) * 128],
                            qq[64:128, nch * 512:(nch + 1) * 512])
                        ptp = sb.tile([128, 1024], BF16,
                                      name=f"pt_b{b}_{hp}_{m}_{nch}",
                                      tag="ptp", bufs=2)
                        nc.scalar.activation(ptp[:], stp[:], EXP,
                                             scale=float(SCALE))
                        if prev is not None:
                            emit_pv(*prev)
                        prev = (m, ptp)
                        pump(650)
                    emit_pv(*prev)

                    # evacuate + normalize
                    for tagc, pv, r0 in (("A", pvA, 0), ("B", pvB, 64)):
                        pvs = sb.tile([HD + 1, 512], F32,
                                      name=f"pvs{tagc}_b{b}_{hp}_{nch}",
                                      tag=f"pvs{tagc}")
                        nc.scalar.activation(pvs[:], pv[:], CPY)
                        dr = sb.tile([1, 512], F32,
                                     name=f"dr{tagc}_b{b}_{hp}_{nch}",
                                     tag=f"dr{tagc}")
                        nc.vector.tensor_copy(dr[:], pvs[64:65, :])
                        rr = sb.tile([1, 512], F32,
                                     name=f"rr{tagc}_b{b}_{hp}_{nch}",
                                     tag=f"rr{tagc}")
                        nc.vector.reciprocal_approx_fast(rr[:], dr[:])
                        rb = sb.tile([64, 512], F32,
                                     name=f"rb{tagc}_b{b}_{hp}_{nch}",
                                     tag=f"rb{tagc}")
                        nc.gpsimd.partition_broadcast(rb[:], rr[:])
                        nc.vector.tensor_tensor(
                            outT[(b, hp)][r0:r0 + 64,
                                          nch * 512:(nch + 1) * 512],
                            pvs[0:64, :], rb[:], mul)
                    pump(600)

            # ----------------- main schedule -----------------
            add_xdma(0)
            add_v_chunk(0, 0)
            pump(360)        # issue x/w DMAs ahead of the const DMAs
            emit_consts()
            add_kq_chunk(0, 1, 0, kT)
            add_kq_chunk(0, 0, 0, qT)
            pump_all()

            add_v_chunk(0, 1)
            add_kq_chunk(0, 1, 1, kT)
            add_kq_chunk(0, 0, 1, qT)
            for hp in range(NHP):
                if hp == 3:
                    add_xdma(1)
                    add_kq_chunk(1, 1, 0, kT)
                elif hp == 6:
                    add_kq_chunk(1, 0, 0, qT)
                attention(0, hp)
            flush_pending()
            pump_all()

            add_v_chunk(1, 0)
            flush_pending()
            pump_all()

            add_v_chunk(1, 1)
            add_kq_chunk(1, 1, 1, kT)
            add_kq_chunk(1, 0, 1, qT)
            for hp in range(NHP):
                if hp == 3:
                    add_proj(0, (0,))
                elif hp == 5:
                    add_proj(0, (1,))
                attention(1, hp)
            flush_pending()
            pump_all()

            add_proj(1, wbig=True)
            pump_all()

    nc.compile()
    return nc


_NC_CACHE = None


def kernel(x, w_qkv, w_proj, b_proj):
    global _NC_CACHE, last_exec_time_ns
    import ml_dtypes

    x = np.ascontiguousarray(np.asarray(x, np.float32))
    w_qkv = np.asarray(w_qkv, np.float32)
    w_proj = np.asarray(w_proj, np.float32)
    b_proj = np.asarray(b_proj, np.float32)

    if _NC_CACHE is None:
        _NC_CACHE = _build()
    nc = _NC_CACHE

    cos_h, sin_h = _freq_tables()
    perm = _qk_perm()
    wq_p = np.empty_like(w_qkv)
    wq_p[0:DIM] = w_qkv[0:DIM][perm]            # q rows permuted
    wq_p[DIM:2 * DIM] = w_qkv[DIM:2 * DIM][perm]  # k rows permuted
    wq_p[2 * DIM:] = w_qkv[2 * DIM:]            # v rows unchanged
    wqkvT = np.ascontiguousarray(wq_p.T).astype(ml_dtypes.bfloat16)
    wprojT = np.ascontiguousarray(w_proj.T).astype(ml_dtypes.bfloat16)
    bproj16 = b_proj.reshape(1, DIM).astype(ml_dtypes.bfloat16)
    # pre-arrange to the [128, NT*16] sbuf layout: [p, t*16+c] = table[t*128+p, c]
    cos16 = np.ascontiguousarray(
        cos_h.reshape(NT, 128, 16).transpose(1, 0, 2).reshape(128, NT * 16)
    ).astype(ml_dtypes.bfloat16)
    sin16 = np.ascontiguousarray(
        sin_h.reshape(NT, 128, 16).transpose(1, 0, 2).reshape(128, NT * 16)
    ).astype(ml_dtypes.bfloat16)
    ident = np.eye(128, dtype=np.float32).astype(ml_dtypes.bfloat16)

    in_maps = []
    for c in range(NCORES):
        xs = x[c * BPC:(c + 1) * BPC]                       # [2, N, DIM]
        xT = np.ascontiguousarray(xs.transpose(0, 2, 1)).astype(
            ml_dtypes.bfloat16)
        in_maps.append({
            "xT": xT, "wqkvT": wqkvT, "wprojT": wprojT,
            "bproj": bproj16, "cosh": cos16, "sinh": sin16,
            "ident": ident,
        })

    trace = bool(os.environ.get("KERNEL_TRACE"))
    kwargs = {}
    if trace:
        kwargs["trace"] = True
        td = os.environ.get("KERNEL_TRACE_DIR")
        if td:
            kwargs["tmpdir"] = td
    res = bass_utils.run_bass_kernel_spmd(
        nc, in_maps, core_ids=list(range(NCORES)), **kwargs)
    last_exec_time_ns = res.exec_time_ns
    out = np.concatenate([res.results[c]["y"] for c in range(NCORES)], axis=0)
    return np.ascontiguousarray(out.reshape(B, N, DIM).astype(np.float32))


if __name__ == "__main__":
    rng = np.random.default_rng(0)
    xs = rng.standard_normal((B, N, DIM), dtype=np.float32)
    wq = rng.standard_normal((3 * DIM, DIM), dtype=np.float32) / 32
    wp = rng.standard_normal((DIM, DIM), dtype=np.float32) / 32
    bp = np.zeros(DIM, np.float32)
    y = kernel(xs, wq, wp, bp)
    print("y", y.shape, y.dtype, float(np.abs(y).max()))
